# revision 1
# baseline (speedup 1.0000x reference)
"""Depth rasterization (MANO hand z-buffer @ 640x640 -> bilinear 128x128).

Key identities exploited:
  * jax.image.resize(640->128, linear, antialias=False) samples input coords
    5*j + 2.0 exactly -> output[i, j] == raster[5i+2, 5j+2]. Only the 128x128
    decimated pixel grid (centers x = 5j+2.5, y = 5i+2.5) is rasterized: a
    25x reduction vs the reference's 640x640 raster.
  * Edge functions and barycentric depth are affine in pixel coords, so each
    triangle yields four planes over the basis (j, i, 1):
      P_k = OFF - S * sign(area) * e_k     (k = 0,1,2 penalty planes)
      W   = (e0*z0 + e1*z1 + e2*z2) / area (depth plane)
    key(p, f) = max(P0, P1, P2, W) equals the interpolated depth when p is
    inside triangle f and is >= OFF (>> the 100 clamp) outside; the z-buffer
    is zbuf(p) = min(100, min_f key(p, f)).
  * Plane evaluation is a K=9 bf16 matmul (coefficients split into 3 bf16
    limbs; the (j, i, 1) basis is exact in bf16, giving fp32-grade accuracy
    at bf16 PE speed); planes are pair-merged as comp-A = [P0|W] and
    comp-B = [P1|P2] streams evaluated on alternating PE row-groups.
  * Per 16x8-pixel tile, candidates are bbox-filtered and hierarchical-z
    pruned on the host (exact: a candidate whose minimum possible depth over
    the tile exceeds the best fully-covering candidate's maximum depth can
    never win). Tiles are chunked to <=256 candidates per work item (host
    min-merges chunks), items are rank-parity balanced across each batch's
    two cores, and slot capacities are per-rank maxima across all 8 cores -
    exact for any input, no truncation.
  * DVE work per slot is 3 element passes: one wide tensor_tensor max
    (u = max(compA, compB)) and a custom fused DVE op
    (out = max(u_lo, u_hi); accum = min-reduce seeded at 100).

Sharding: 8 cores; each batch element's 128 tiles split across 2 cores.
"""

import numpy as np
import ml_dtypes

import concourse.bacc as bacc
import concourse.mybir as mybir
import concourse.tile as tile
from concourse.bass_utils import run_bass_kernel_spmd

_B, _V, _F = 4, 778, 1538
_H = _W = 128
_TJ, _TI = 16, 8   # tile size in output pixels (x, y)
_NTILE = (_H // _TI) * (_W // _TJ)  # 128 tiles per batch image
_WMAX = 256        # max slot width (pair-merged 2w <= 512 = one PSUM bank)
_OFF = 1000.0      # penalty-plane offset (>> 100 clamp)
_S = 1.0e9         # penalty scale
_BIGC = 1.0e7      # plane constant for padding/invalid
_CLAMP = 100.0
_COVER_MARGIN = 1.0    # e*s margin (e-units) for the full-cover test
_BOUND_MARGIN = 1e-3   # depth margin for the prune bound

_F32 = mybir.dt.float32
_BF16 = mybir.dt.bfloat16
_BF16_NP = ml_dtypes.bfloat16

_NC_CACHE = {}
_OP_CACHE = {}
PROFILE = {}


def _maxpair_minred_op():
    """Custom DVE op: out = max(in0, in1); accum_out = min(out) seeded s0."""
    if "op" in _OP_CACHE:
        return _OP_CACHE["op"]
    import concourse.dve_ops as dve_ops
    from concourse.dve_spec import C0, Spec, Src0, Src1, lower, maxx, minn
    from concourse.dve_table_gen import dve_ver_for
    from concourse.dve_uop import DveOpSpec

    name = "MAXPAIR_MINRED_ANT"
    for op in dve_ops.OPS:
        if op.name == name:
            _OP_CACHE["op"] = op
            return op
    spec = Spec(body=maxx(Src0, Src1), accum=minn, accum_init=C0)
    opcode = dve_ops._CUSTOM_DVE_ROW_BASE + len(dve_ops.OPS)
    assert opcode < 0x20
    dve_ops._SUB_OPCODE_FOR_NAME[name] = opcode
    ver = dve_ver_for("TRN2")
    sha = DveOpSpec(name=name, opcode=opcode, uops=lower(spec, ver=ver),
                    rd1_en=True).sha(ver)
    op = dve_ops.DveOp(name, spec, subdim=False, uops_sha={ver: sha})
    dve_ops.OPS.append(op)
    dve_ops.CUSTOM_DVE_SPECS[name] = spec
    _OP_CACHE["op"] = op
    return op


def _build_nc(caps, groups):
    """caps: per-slot widths w (32-granular, <= _WMAX); groups: ((w, k), ...)
    of consecutive equal-width slots with 2*k*w <= 512 (one PSUM bank)."""
    nslot = len(caps)
    total2 = 2 * int(sum(caps))
    op = _maxpair_minred_op()
    nc = bacc.Bacc("TRN2", target_bir_lowering=False, debug=False, num_devices=8)
    # dense [128, ...] input: pair-merged coef streams (comp-A = [P0|W] limbs
    # at partitions 0-8 & 64-72, comp-B = [P1|P2] at 32-40 & 96-104), then
    # nslot*128 pixel-basis cols at all four row-groups.
    data_d = nc.dram_tensor("data", [128, total2 + nslot * 128], _BF16, kind="ExternalInput")
    out_d = nc.dram_tensor("out", [128, nslot], _F32, kind="ExternalOutput")

    with tile.TileContext(nc) as tc:
        with (
            tc.tile_pool(name="const", bufs=1) as cpool,
            tc.tile_pool(name="scr", bufs=6) as spool,
            tc.tile_pool(name="ps", bufs=8, space="PSUM") as ppool,
        ):
            zmin = cpool.tile([128, nslot], _F32)
            # coef DMA in ~6 chunks at group boundaries; pix in 4 chunks
            goff = [0]
            for w, k in groups:
                goff.append(goff[-1] + 2 * w * k)
            # chunk boundaries (in groups): fine-grained early so the first
            # compute groups start as soon as their data lands
            gb = [0, 1, 2, 4, 6, 9, 13, 18, 24]
            gb = sorted({min(g, len(groups)) for g in gb} | {len(groups)})
            slot_of_group = [0]
            for w, k in groups:
                slot_of_group.append(slot_of_group[-1] + k)
            ctiles = []  # (col range, tile)
            ptiles = []  # (slot range, tile)
            dmas = []
            for i in range(len(gb) - 1):
                c0, c1 = goff[gb[i]], goff[gb[i + 1]]
                s0, s1 = slot_of_group[gb[i]], slot_of_group[gb[i + 1]]
                if c1 > c0:
                    ct = cpool.tile([128, c1 - c0], _BF16, name=f"coef{i}")
                    ctiles.append((c0, c1, ct))
                    dmas.append((ct, data_d.ap()[:, c0:c1]))
                if s1 > s0:
                    pt = cpool.tile([128, (s1 - s0) * 128], _BF16, name=f"pix{i}")
                    ptiles.append((s0, s1, pt))
                    dmas.append((pt, data_d.ap()[:, total2 + s0 * 128 : total2 + s1 * 128]))
            for dst, srcap in dmas:
                nc.sync.dma_start(dst[:], srcap)

            def coef_view(c0, c1):
                for t0, t1, ct in ctiles:
                    if t0 <= c0 and c1 <= t1:
                        return ct[:, c0 - t0 : c1 - t0]
                raise AssertionError((c0, c1))

            def pix_view(s):
                for s0, s1, pt in ptiles:
                    if s0 <= s < s1:
                        return pt[:, (s - s0) * 128 : (s - s0 + 1) * 128]
                raise AssertionError(s)

            gbase = 0
            for gi, (w, k) in enumerate(groups):
                kw2 = 2 * w * k
                go = goff[gi]
                pa = ppool.tile([128, 512], _F32, tag="ps", name="pa")
                pb = ppool.tile([128, 512], _F32, tag="ps", name="pb")
                for q in range(k):
                    s = gbase + q
                    o = 2 * w * q
                    ra, rb = (0, 32) if gi % 2 == 0 else (64, 96)
                    pv = pix_view(s)
                    cv = coef_view(go + o, go + o + 2 * w)
                    nc.tensor.matmul(pa[:, o : o + 2 * w], pv[ra : ra + 9, :],
                                     cv[ra : ra + 9, :],
                                     start=True, stop=True, tile_position=(ra, 0))
                    nc.tensor.matmul(pb[:, o : o + 2 * w], pv[rb : rb + 9, :],
                                     cv[rb : rb + 9, :],
                                     start=True, stop=True, tile_position=(rb, 0))
                # ScalarE pulls comp-A to SBUF (DVE reads max one PSUM operand)
                ta = spool.tile([128, 512], _F32, tag="ta", name="ta")
                nc.scalar.copy(ta[:, :kw2], pa[:, :kw2])
                u = spool.tile([128, 512], _F32, tag="u", name="u")
                nc.vector.tensor_tensor(u[:, :kw2], ta[:, :kw2], pb[:, :kw2],
                                        op=mybir.AluOpType.max)
                for q in range(k):
                    s = gbase + q
                    o = 2 * w * q
                    keyt = spool.tile([128, 256], _F32, tag="key", name="keyt")
                    if PROFILE.get("no_custom"):
                        nc.vector.tensor_tensor(keyt[:, :w], u[:, o : o + w],
                                                u[:, o + w : o + 2 * w],
                                                op=mybir.AluOpType.max)
                        nc.vector.tensor_reduce(zmin[:, s : s + 1], keyt[:, :w],
                                                axis=mybir.AxisListType.X,
                                                op=mybir.AluOpType.min)
                    else:
                        nc.vector._custom_dve(
                            op,
                            out=keyt[:, :w],
                            in0=u[:, o : o + w],
                            in1=u[:, o + w : o + 2 * w],
                            s0=_CLAMP,
                            accum_out=zmin[:, s : s + 1],
                        )
                gbase += k

            nc.sync.dma_start(out_d.ap(), zmin[:])

    nc.compile()
    return nc


def _get_nc(caps, groups):
    key = (caps, groups)
    if key not in _NC_CACHE:
        _NC_CACHE[key] = _build_nc(caps, groups)
    return _NC_CACHE[key]


def _planes64(vertices, faces):
    """Full-precision planes on basis (j, i, 1): [B, 4, 3, F] f64 + aux."""
    v64 = vertices.astype(np.float64)
    fidx = np.asarray(faces).astype(np.int64).reshape(-1)
    fv = v64[:, fidx, :].reshape(_B, _F, 3, 3)
    x0, y0, z0 = fv[:, :, 0, 0], fv[:, :, 0, 1], fv[:, :, 0, 2]
    x1, y1, z1 = fv[:, :, 1, 0], fv[:, :, 1, 1], fv[:, :, 1, 2]
    x2, y2, z2 = fv[:, :, 2, 0], fv[:, :, 2, 1], fv[:, :, 2, 2]

    # area exactly as the reference computes it (float32 ops)
    v32 = vertices.astype(np.float32)
    fv32 = v32[:, fidx, :].reshape(_B, _F, 3, 3)
    xa, ya = fv32[:, :, 0, 0], fv32[:, :, 0, 1]
    xb, yb = fv32[:, :, 1, 0], fv32[:, :, 1, 1]
    xc, yc = fv32[:, :, 2, 0], fv32[:, :, 2, 1]
    area32 = (xb - xa) * (yc - ya) - (yb - ya) * (xc - xa)
    s = np.sign(area32).astype(np.float64)
    valid = np.abs(area32) > 1e-12

    A0 = -(y2 - y1); B0 = x2 - x1; C0 = (y2 - y1) * x1 - (x2 - x1) * y1
    A1 = -(y0 - y2); B1 = x0 - x2; C1 = (y0 - y2) * x2 - (x0 - x2) * y2
    A2 = -(y1 - y0); B2 = x1 - x0; C2 = (y1 - y0) * x0 - (x1 - x0) * y0

    area64 = np.where(valid, area32.astype(np.float64), 1.0)
    Aw = (z0 * A0 + z1 * A1 + z2 * A2) / area64
    Bw = (z0 * B0 + z1 * B1 + z2 * B2) / area64
    Cw = (z0 * C0 + z1 * C1 + z2 * C2) / area64

    planes = np.zeros((_B, 4, 3, _F), np.float64)
    raw = [
        (-_S * s * A0, -_S * s * B0, _OFF - _S * s * C0),
        (-_S * s * A1, -_S * s * B1, _OFF - _S * s * C1),
        (-_S * s * A2, -_S * s * B2, _OFF - _S * s * C2),
        (Aw, Bw, Cw),
    ]
    for k, (a, b, c) in enumerate(raw):
        a = np.where(valid, a, 0.0)
        b = np.where(valid, b, 0.0)
        c = np.where(valid, c, _BIGC)
        # basis change px = 5j + 2.5, py = 5i + 2.5 -> (j, i, 1)
        planes[:, k, 0] = 5.0 * a
        planes[:, k, 1] = 5.0 * b
        planes[:, k, 2] = 2.5 * a + 2.5 * b + c

    xsmin = fv[..., 0].min(2); xsmax = fv[..., 0].max(2)
    ysmin = fv[..., 1].min(2); ysmax = fv[..., 1].max(2)
    zmin_tri = fv[..., 2].min(2)
    return planes, valid, xsmin, xsmax, ysmin, ysmax, zmin_tri


def _split3(c64):
    hi = c64.astype(_BF16_NP).astype(np.float64)
    mid = (c64 - hi).astype(_BF16_NP).astype(np.float64)
    lo = (c64 - hi - mid).astype(_BF16_NP)
    return hi.astype(_BF16_NP), mid.astype(_BF16_NP), lo


def _prepare(vertices, faces):
    planes, valid, xsmin, xsmax, ysmin, ysmax, zmin_tri = _planes64(vertices, faces)
    ntj = _W // _TJ

    # prune per tile, chunk to <=_WMAX, rank-parity balance across all 8
    # cores (a core may hold tiles of any batch - the coef stream is data)
    core_items = [[] for _ in range(8)]  # items: (batch, tile_t, cand_idx_array)
    all_items = []
    for b in range(_B):
        P = planes[b]
        items = all_items
        for t in range(_NTILE):
            tj, ti = t % ntj, t // ntj
            j0, i0 = tj * _TJ, ti * _TI
            xlo, xhi = 5 * j0 + 2.5, 5 * (j0 + _TJ - 1) + 2.5
            ylo, yhi = 5 * i0 + 2.5, 5 * (i0 + _TI - 1) + 2.5
            cand = np.where(valid[b] & (xsmax[b] >= xlo) & (xsmin[b] <= xhi)
                            & (ysmax[b] >= ylo) & (ysmin[b] <= yhi))[0]
            if len(cand):
                corners = np.array(
                    [[j0, i0, 1], [j0 + _TJ - 1, i0, 1],
                     [j0, i0 + _TI - 1, 1], [j0 + _TJ - 1, i0 + _TI - 1, 1]],
                    np.float64)
                Wc = corners @ P[3][:, cand]
                zlo = np.maximum(Wc.min(0), zmin_tri[b][cand])
                covers = np.ones(len(cand), bool)
                for k in range(3):
                    Pc = corners @ P[k][:, cand]
                    covers &= (Pc <= _OFF - _S * _COVER_MARGIN).all(axis=0)
                bound = (Wc.max(0)[covers].min() + _BOUND_MARGIN) if covers.any() else np.inf
                keep = zlo <= bound
                order = cand[keep][np.argsort(zlo[keep])]
            else:
                order = cand
            if len(order) == 0:
                items.append((b, t, order))
            else:
                for c0 in range(0, len(order), _WMAX):
                    items.append((b, t, order[c0 : c0 + _WMAX]))
    all_items.sort(key=lambda it: -len(it[2]))
    for r, it in enumerate(all_items):
        core_items[r % 8].append(it)

    nslot = max(len(ci) for ci in core_items)
    rawcaps = []
    for s in range(nslot):
        m = max((len(ci[s][2]) if s < len(ci) else 0) for ci in core_items)
        rawcaps.append(max(16, ((m + 15) // 16) * 16))

    # groups of consecutive slots padded to the group's (max) width, with
    # pair-merged group width 2*k*w <= 512 (one PSUM bank)
    groups = []
    s = 0
    while s < nslot:
        w = rawcaps[s]
        k = 1
        while s + k < nslot and 2 * (k + 1) * w <= 512:
            k += 1
        groups.append((w, k))
        s += k
    groups = tuple(groups)
    caps = []
    for w, k in groups:
        caps.extend([w] * k)
    caps = tuple(caps)
    total2 = 2 * sum(caps)

    in_maps = []
    for c in range(8):
        items = core_items[c]
        compA = np.zeros((3, total2), np.float64)
        compB = np.zeros((3, total2), np.float64)
        compA[2, :] = _BIGC
        compB[2, :] = _BIGC
        pix_g = np.zeros((3, nslot * 128), np.float32)
        off = 0
        for s in range(nslot):
            w = caps[s]
            jj = ii = np.zeros(128, np.float32)
            if s < len(items):
                b, t, idx = items[s]
                n = len(idx)
                compA[:, off : off + n] = planes[b, 0][:, idx]          # P0
                compA[:, off + w : off + w + n] = planes[b, 3][:, idx]  # W
                compB[:, off : off + n] = planes[b, 1][:, idx]          # P1
                compB[:, off + w : off + w + n] = planes[b, 2][:, idx]  # P2
                tj, ti = t % ntj, t // ntj
                j0, i0 = tj * _TJ, ti * _TI
                jj = j0 + np.tile(np.arange(_TJ, dtype=np.float32), _TI)
                ii = i0 + np.repeat(np.arange(_TI, dtype=np.float32), _TJ)
            off += 2 * w
            pix_g[0, s * 128 : (s + 1) * 128] = jj
            pix_g[1, s * 128 : (s + 1) * 128] = ii
            pix_g[2, s * 128 : (s + 1) * 128] = 1.0
        data = np.zeros((128, total2 + nslot * 128), _BF16_NP)
        for comp, bases in ((compA, (0, 64)), (compB, (32, 96))):
            hi, mid, lo = _split3(comp)
            for base in bases:
                data[base + 0 : base + 3, :total2] = hi
                data[base + 3 : base + 6, :total2] = mid
                data[base + 6 : base + 9, :total2] = lo
        pix16 = np.vstack([pix_g, pix_g, pix_g]).astype(_BF16_NP)
        for base in (0, 32, 64, 96):
            data[base : base + 9, total2:] = pix16
        in_maps.append({"data": data})
    return caps, groups, in_maps, core_items


def kernel(vertices, faces):
    vertices = np.asarray(vertices)
    faces = np.asarray(faces)
    caps, groups, in_maps, core_items = _prepare(vertices, faces)

    nc = _get_nc(caps, groups)
    kw = dict(PROFILE.get("run_kwargs", {}))
    res = run_bass_kernel_spmd(nc, in_maps, list(range(8)), **kw)
    PROFILE["last_result"] = res

    ntj = _W // _TJ
    out = np.full((_B, _H, _W), _CLAMP, np.float32)
    for c in range(8):
        z = res.results[c]["out"]  # [128, nslot]
        for s, (b, t, idx) in enumerate(core_items[c]):
            tj, ti = t % ntj, t // ntj
            j0, i0 = tj * _TJ, ti * _TI
            blk = z[:, s].reshape(_TI, _TJ)
            out[b, i0 : i0 + _TI, j0 : j0 + _TJ] = np.minimum(
                out[b, i0 : i0 + _TI, j0 : j0 + _TJ], blk)
    return out



# revision 5
# speedup vs baseline: 2.0953x; 2.0953x over previous
"""Depth rasterization (MANO hand z-buffer @ 640x640 -> bilinear 128x128).

Key identities exploited:
  * jax.image.resize(640->128, linear, antialias=False) samples input coords
    5*j + 2.0 exactly -> output[i, j] == raster[5i+2, 5j+2]. Only the 128x128
    decimated pixel grid (centers x = 5j+2.5, y = 5i+2.5) is rasterized: a
    25x reduction vs the reference's 640x640 raster.
  * Edge functions and barycentric depth are affine in pixel coords, so each
    triangle yields four planes over the local basis (jl, il, 1) of its tile
    (tile origin folded into the constant term on host):
      P_k = OFF - S * sign(area) * e_k     (k = 0,1,2 penalty planes)
      W   = (e0*z0 + e1*z1 + e2*z2) / area (depth plane)
    key(p, f) = max(P0, P1, P2, W) equals the interpolated depth when p is
    inside triangle f and is >= OFF (>> the 100 clamp) outside; the z-buffer
    is zbuf(p) = min(100, min_f key(p, f)).
  * Plane evaluation is a K=9 bf16 matmul (coefficients split into 3 bf16
    limbs; the (jl, il, 1) basis is exact in bf16, giving fp32-grade accuracy
    at bf16 PE speed). Per slot of width w the PE writes [P0|W] (bank A,
    quadrant ra) and [P1|P2] (bank B, quadrant rb); ScalarE pulls bank A to
    SBUF (DVE reads at most one PSUM operand), DVE takes u = max(ta, pb)
    (bf16 out), pair-max within each slot (3D APs, one instr per group), and
    min-reduces over candidates (axis=X over [128, k, w]).
  * Per 16x8-pixel tile, candidates are bbox-filtered, then pruned exactly on
    a 4x4 subtile grid: a candidate is dropped if in every subtile it either
    misses the subtile entirely (some edge has all 4 corners outside by a
    margin safely above the reference's own fp32 edge-function noise) or its
    minimum possible depth exceeds the subtile's best fully-covering
    candidate's maximum depth. Exact vs the reference for any input.
  * Only the 9 coefficient rows actually read by the PE are shipped (HBM
    [9, cols]); the pixel basis is one shared [9, 128] tensor replicated into
    the 4 PE quadrants on device.

Sharding: 8 cores; (batch, tile) work items rank-balanced across all cores.
"""

import numpy as np
import ml_dtypes

import concourse.bacc as bacc
import concourse.mybir as mybir
import concourse.tile as tile
from concourse.bass_utils import run_bass_kernel_spmd

_B, _V, _F = 4, 778, 1538
_H = _W = 128
_TJ, _TI = 16, 8   # tile size in output pixels (x, y)
_NTILE = (_H // _TI) * (_W // _TJ)  # 128 tiles per batch image
_SUBJ, _SUBI = 4, 4  # subtile grid for host pruning
_WMAX = 256        # max slot width
_GRAN = 4          # slot width granularity
_OFF = 1000.0      # penalty-plane offset (>> 100 clamp)
_S = 1.0e9         # penalty scale
_BIGC = 1.0e7      # plane constant for padding/invalid
_CLAMP = 100.0
_COVER_MARGIN = 1.0    # e*s margin (e-units) for the full-cover test
_REJ_MARGIN = 1.0      # e*s margin (e-units) for exact edge rejection
_BOUND_MARGIN = 1e-3   # depth margin for the prune bound

_F32 = mybir.dt.float32
_BF16 = mybir.dt.bfloat16
_BF16_NP = ml_dtypes.bfloat16

_NC_CACHE = {}
PROFILE = {}


def _build_nc(groups):
    """groups: ((w, k), ...) consecutive slots padded to width w, k per
    group, with 2*w*k <= 512 (one PSUM bank per component pair)."""
    nslot = sum(k for _, k in groups)
    total4 = sum(4 * w * k for w, k in groups)
    nc = bacc.Bacc("TRN2", target_bir_lowering=False, debug=False, num_devices=8)
    coef_d = nc.dram_tensor("coef", [9, total4], _BF16, kind="ExternalInput")
    pix_d = nc.dram_tensor("pix", [9, 128], _BF16, kind="ExternalInput")
    out_d = nc.dram_tensor("out", [128, nslot], _BF16, kind="ExternalOutput")

    with tile.TileContext(nc) as tc:
        with (
            tc.tile_pool(name="const", bufs=1) as cpool,
            tc.tile_pool(name="scr", bufs=6) as spool,
            tc.tile_pool(name="ps", bufs=4, space="PSUM") as ppool,
        ):
            pix_t = cpool.tile([128, 128], _BF16, name="pix")
            for ra in (0, 32, 64, 96):
                nc.sync.dma_start(pix_t[ra : ra + 9, :], pix_d.ap()[0:9, :])
            zmin = cpool.tile([128, nslot], _BF16, name="zmin")

            off = 0
            s0 = 0
            for gi, (w, k) in enumerate(groups):
                wk = w * k
                ra, rb = (0, 32) if gi % 2 == 0 else (64, 96)
                ctA = cpool.tile([128, 2 * wk], _BF16, name=f"cA{gi}")
                ctB = cpool.tile([128, 2 * wk], _BF16, name=f"cB{gi}")
                nc.sync.dma_start(ctA[ra : ra + 9, :],
                                  coef_d.ap()[0:9, off : off + 2 * wk])
                nc.sync.dma_start(ctB[rb : rb + 9, :],
                                  coef_d.ap()[0:9, off + 2 * wk : off + 4 * wk])
                pa = ppool.tile([128, 2 * wk], _F32, tag="pa", name="pa")
                pb = ppool.tile([128, 2 * wk], _F32, tag="pb", name="pb")
                nc.tensor.matmul(pa[:, :], pix_t[ra : ra + 9, :],
                                 ctA[ra : ra + 9, :],
                                 start=True, stop=True, tile_position=(ra, 0))
                nc.tensor.matmul(pb[:, :], pix_t[rb : rb + 9, :],
                                 ctB[rb : rb + 9, :],
                                 start=True, stop=True, tile_position=(rb, 0))
                # ScalarE pulls comp-A to SBUF (DVE reads max one PSUM operand)
                ta = spool.tile([128, 2 * wk], _F32, tag="ta", name="ta")
                nc.scalar.copy(ta[:, :], pa[:, :])
                u = spool.tile([128, 2 * wk], _BF16, tag="u", name="u")
                nc.vector.tensor_tensor(u[:, :], ta[:, :], pb[:, :],
                                        op=mybir.AluOpType.max)
                key = spool.tile([128, wk], _BF16, tag="key", name="key")
                u3 = u[:].rearrange("p (k x) -> p k x", x=2 * w)
                k3 = key[:].rearrange("p (k x) -> p k x", x=w)
                nc.vector.tensor_tensor(k3, u3[:, :, 0:w], u3[:, :, w : 2 * w],
                                        op=mybir.AluOpType.max)
                nc.vector.tensor_reduce(zmin[:, s0 : s0 + k], k3,
                                        axis=mybir.AxisListType.X,
                                        op=mybir.AluOpType.min)
                off += 4 * wk
                s0 += k

            nc.sync.dma_start(out_d.ap(), zmin[:])

    nc.compile()
    return nc


def _get_nc(groups):
    if groups not in _NC_CACHE:
        _NC_CACHE[groups] = _build_nc(groups)
    return _NC_CACHE[groups]


def _planes64(vertices, faces):
    """Full-precision planes on global basis (j, i, 1): [B, 4, 3, F] f64."""
    v64 = vertices.astype(np.float64)
    fidx = np.asarray(faces).astype(np.int64).reshape(-1)
    fv = v64[:, fidx, :].reshape(_B, _F, 3, 3)
    x0, y0, z0 = fv[:, :, 0, 0], fv[:, :, 0, 1], fv[:, :, 0, 2]
    x1, y1, z1 = fv[:, :, 1, 0], fv[:, :, 1, 1], fv[:, :, 1, 2]
    x2, y2, z2 = fv[:, :, 2, 0], fv[:, :, 2, 1], fv[:, :, 2, 2]

    # area exactly as the reference computes it (float32 ops)
    v32 = vertices.astype(np.float32)
    fv32 = v32[:, fidx, :].reshape(_B, _F, 3, 3)
    xa, ya = fv32[:, :, 0, 0], fv32[:, :, 0, 1]
    xb, yb = fv32[:, :, 1, 0], fv32[:, :, 1, 1]
    xc, yc = fv32[:, :, 2, 0], fv32[:, :, 2, 1]
    area32 = (xb - xa) * (yc - ya) - (yb - ya) * (xc - xa)
    s = np.sign(area32).astype(np.float64)
    valid = np.abs(area32) > 1e-12

    A0 = -(y2 - y1); B0 = x2 - x1; C0 = (y2 - y1) * x1 - (x2 - x1) * y1
    A1 = -(y0 - y2); B1 = x0 - x2; C1 = (y0 - y2) * x2 - (x0 - x2) * y2
    A2 = -(y1 - y0); B2 = x1 - x0; C2 = (y1 - y0) * x0 - (x1 - x0) * y0

    area64 = np.where(valid, area32.astype(np.float64), 1.0)
    Aw = (z0 * A0 + z1 * A1 + z2 * A2) / area64
    Bw = (z0 * B0 + z1 * B1 + z2 * B2) / area64
    Cw = (z0 * C0 + z1 * C1 + z2 * C2) / area64

    planes = np.zeros((_B, 4, 3, _F), np.float64)
    raw = [
        (-_S * s * A0, -_S * s * B0, _OFF - _S * s * C0),
        (-_S * s * A1, -_S * s * B1, _OFF - _S * s * C1),
        (-_S * s * A2, -_S * s * B2, _OFF - _S * s * C2),
        (Aw, Bw, Cw),
    ]
    for k, (a, b, c) in enumerate(raw):
        a = np.where(valid, a, 0.0)
        b = np.where(valid, b, 0.0)
        c = np.where(valid, c, _BIGC)
        # basis change px = 5j + 2.5, py = 5i + 2.5 -> (j, i, 1)
        planes[:, k, 0] = 5.0 * a
        planes[:, k, 1] = 5.0 * b
        planes[:, k, 2] = 2.5 * a + 2.5 * b + c

    xsmin = fv[..., 0].min(2); xsmax = fv[..., 0].max(2)
    ysmin = fv[..., 1].min(2); ysmax = fv[..., 1].max(2)
    zmin_tri = fv[..., 2].min(2)
    return planes, valid, xsmin, xsmax, ysmin, ysmax, zmin_tri


def _split3(c64):
    hi = c64.astype(_BF16_NP).astype(np.float64)
    mid = (c64 - hi).astype(_BF16_NP).astype(np.float64)
    lo = (c64 - hi - mid).astype(_BF16_NP)
    return hi.astype(_BF16_NP), mid.astype(_BF16_NP), lo


def _corners(j0, i0, nj, ni):
    return np.array([[j0, i0, 1], [j0 + nj - 1, i0, 1],
                     [j0, i0 + ni - 1, 1], [j0 + nj - 1, i0 + ni - 1, 1]],
                    np.float64)


def _prepare(vertices, faces):
    planes, valid, xsmin, xsmax, ysmin, ysmax, zmin_tri = _planes64(vertices, faces)
    ntj = _W // _TJ
    ssj, ssi = _TJ // _SUBJ, _TI // _SUBI

    all_items = []
    for b in range(_B):
        P = planes[b]
        for t in range(_NTILE):
            tj, ti = t % ntj, t // ntj
            j0, i0 = tj * _TJ, ti * _TI
            xlo, xhi = 5 * j0 + 2.5, 5 * (j0 + _TJ - 1) + 2.5
            ylo, yhi = 5 * i0 + 2.5, 5 * (i0 + _TI - 1) + 2.5
            cand = np.where(valid[b] & (xsmax[b] >= xlo) & (xsmin[b] <= xhi)
                            & (ysmax[b] >= ylo) & (ysmin[b] <= yhi))[0]
            if len(cand) == 0:
                continue
            # exact subtile prune: keep a candidate iff it can win somewhere
            keep_any = np.zeros(len(cand), bool)
            Pk = [P[k][:, cand] for k in range(4)]
            zt = zmin_tri[b][cand]
            for sj in range(_SUBJ):
                for si in range(_SUBI):
                    C = _corners(j0 + sj * ssj, i0 + si * ssi, ssj, ssi)
                    alive = np.ones(len(cand), bool)
                    for k in range(3):
                        Pc = C @ Pk[k]
                        alive &= ~((Pc >= _OFF + _S * _REJ_MARGIN).all(axis=0))
                    idx = np.where(alive)[0]
                    if len(idx) == 0:
                        continue
                    Wc = C @ Pk[3][:, idx]
                    zlo = np.maximum(Wc.min(0), zt[idx])
                    covers = np.ones(len(idx), bool)
                    for k in range(3):
                        Pc = C @ Pk[k][:, idx]
                        covers &= (Pc <= _OFF - _S * _COVER_MARGIN).all(axis=0)
                    bound = (Wc.max(0)[covers].min() + _BOUND_MARGIN
                             ) if covers.any() else np.inf
                    keep_any[idx[zlo <= bound]] = True
            kept = cand[keep_any]
            if len(kept) == 0:
                continue
            Cf = _corners(j0, i0, _TJ, _TI)
            Wf = Cf @ P[3][:, kept]
            zlo_f = np.maximum(Wf.min(0), zmin_tri[b][kept])
            order = kept[np.argsort(zlo_f)]
            for c0 in range(0, len(order), _WMAX):
                all_items.append((b, t, order[c0 : c0 + _WMAX]))

    all_items.sort(key=lambda it: -len(it[2]))
    core_items = [[] for _ in range(8)]
    for r, it in enumerate(all_items):
        core_items[r % 8].append(it)

    nslot = max(len(ci) for ci in core_items)
    rawcaps = []
    for s in range(nslot):
        m = max((len(ci[s][2]) if s < len(ci) else 0) for ci in core_items)
        rawcaps.append(max(_GRAN, ((m + _GRAN - 1) // _GRAN) * _GRAN))

    groups = []
    s = 0
    while s < nslot:
        w = rawcaps[s]
        k = min(max(1, 512 // (2 * w)), nslot - s)
        groups.append((w, k))
        s += k
    groups = tuple(groups)
    caps = []
    for w, k in groups:
        caps.extend([w] * k)
    total4 = sum(4 * w * k for w, k in groups)

    in_maps = []
    jl = (np.arange(128) % _TJ).astype(np.float32)
    il = (np.arange(128) // _TJ).astype(np.float32)
    pix1 = np.stack([jl, il, np.ones(128, np.float32)])
    pix = np.vstack([pix1, pix1, pix1]).astype(_BF16_NP)
    for c in range(8):
        items = core_items[c]
        stage = np.zeros((3, total4), np.float64)
        stage[2, :] = _BIGC
        off = 0
        s = 0
        for w, k in groups:
            wk = w * k
            for q in range(k):
                if s < len(items):
                    b, t, idx = items[s]
                    n = len(idx)
                    tj, ti = t % ntj, t // ntj
                    j0, i0 = tj * _TJ, ti * _TI
                    Pl = np.empty((4, 3, n))
                    for kk in range(4):
                        a = planes[b, kk, 0, idx]
                        b_ = planes[b, kk, 1, idx]
                        cc = planes[b, kk, 2, idx] + a * j0 + b_ * i0
                        Pl[kk] = np.stack([a, b_, cc])
                    cA = off + 2 * w * q
                    cB = off + 2 * wk + 2 * w * q
                    stage[:, cA : cA + n] = Pl[0]          # P0
                    stage[:, cA + w : cA + w + n] = Pl[3]  # W
                    stage[:, cB : cB + n] = Pl[1]          # P1
                    stage[:, cB + w : cB + w + n] = Pl[2]  # P2
                s += 1
            off += 4 * wk
        hi, mid, lo = _split3(stage)
        coef = np.concatenate([hi, mid, lo], axis=0)
        in_maps.append({"coef": coef, "pix": pix})
    return groups, in_maps, core_items


def kernel(vertices, faces):
    vertices = np.asarray(vertices)
    faces = np.asarray(faces)
    groups, in_maps, core_items = _prepare(vertices, faces)

    nc = _get_nc(groups)
    kw = dict(PROFILE.get("run_kwargs", {}))
    res = run_bass_kernel_spmd(nc, in_maps, list(range(8)), **kw)
    PROFILE["last_result"] = res

    ntj = _W // _TJ
    out = np.full((_B, _H, _W), _CLAMP, np.float32)
    for c in range(8):
        z = np.asarray(res.results[c]["out"]).astype(np.float32)  # [128, nslot]
        for s, (b, t, idx) in enumerate(core_items[c]):
            tj, ti = t % ntj, t // ntj
            j0, i0 = tj * _TJ, ti * _TI
            blk = z[:, s].reshape(_TI, _TJ)
            out[b, i0 : i0 + _TI, j0 : j0 + _TJ] = np.minimum(
                out[b, i0 : i0 + _TI, j0 : j0 + _TJ], blk)
    return out


# revision 8
# speedup vs baseline: 2.5433x; 1.2138x over previous
"""Depth rasterization (MANO hand z-buffer @ 640x640 -> bilinear 128x128).

Key identities exploited:
  * jax.image.resize(640->128, linear, antialias=False) samples input coords
    5*j + 2.0 exactly -> output[i, j] == raster[5i+2, 5j+2]. Only the 128x128
    decimated pixel grid (centers x = 5j+2.5, y = 5i+2.5) is rasterized: a
    25x reduction vs the reference's 640x640 raster.
  * Edge functions and barycentric depth are affine in pixel coords, so each
    triangle yields four planes over the local basis (jl, il, 1) of its tile
    (tile origin folded into the constant term on host):
      P_k = OFF - S * sign(area) * e_k     (k = 0,1,2 penalty planes)
      W   = (e0*z0 + e1*z1 + e2*z2) / area (depth plane)
    key(p, f) = max(P0, P1, P2, W) equals the interpolated depth when p is
    inside triangle f and is >= OFF (>> the 100 clamp) outside; the z-buffer
    is zbuf(p) = min(100, min_f key(p, f)).
  * Plane evaluation is a K=9 bf16 matmul (coefficients split into 3 bf16
    limbs; the (jl, il, 1) basis is exact in bf16, giving fp32-grade accuracy
    at bf16 PE speed). Per slot of width w the PE writes [P0|W] (bank A,
    quadrant ra) and [P1|P2] (bank B, quadrant rb); ScalarE pulls bank A to
    SBUF (DVE reads at most one PSUM operand), DVE takes u = max(ta, pb)
    (bf16 out), pair-max within each slot (3D APs, one instr per group), and
    min-reduces over candidates (axis=X over [128, k, w]).
  * Per 16x8-pixel tile, candidates are bbox-filtered, then pruned exactly on
    a 4x4 subtile grid: a candidate is dropped if in every subtile it either
    misses the subtile entirely (some edge has all 4 corners outside by a
    margin safely above the reference's own fp32 edge-function noise) or its
    minimum possible depth exceeds the subtile's best fully-covering
    candidate's maximum depth. Exact vs the reference for any input.
  * Only the 9 coefficient rows actually read by the PE are shipped (HBM
    [9, cols]); the pixel basis is one shared [9, 128] tensor replicated into
    the 4 PE quadrants on device.

Sharding: 8 cores; (batch, tile) work items rank-balanced across all cores.
"""

import numpy as np
import ml_dtypes

import concourse.bacc as bacc
import concourse.mybir as mybir
import concourse.tile as tile
from concourse.bass_utils import run_bass_kernel_spmd

_B, _V, _F = 4, 778, 1538
_H = _W = 128
_TJ, _TI = 16, 8   # tile size in output pixels (x, y)
_NTILE = (_H // _TI) * (_W // _TJ)  # 128 tiles per batch image
_SUBJ, _SUBI = 4, 4  # subtile grid for host pruning
_WMAX = 256        # max slot width
_GRAN = 4          # slot width granularity
_OFF = 1000.0      # penalty-plane offset (>> 100 clamp)
_S = 1.0e9         # penalty scale
_BIGC = 1.0e7      # plane constant for padding/invalid
_CLAMP = 100.0
_COVER_MARGIN = 1.0    # e*s margin (e-units) for the full-cover test
_REJ_MARGIN = 1.0      # e*s margin (e-units) for exact edge rejection
_BOUND_MARGIN = 1e-3   # depth margin for the prune bound

_F32 = mybir.dt.float32
_BF16 = mybir.dt.bfloat16
_BF16_NP = ml_dtypes.bfloat16

_NC_CACHE = {}
PROFILE = {}


def _region_layout(groups):
    """Column layout of the [18, XT] HBM data tensor: even groups' 2wk-blocks,
    128 pix cols, odd groups' 2wk-blocks, 128 pix cols. Rows 0-8 carry the
    comp-A limbs (quadrants 0/64), rows 9-17 comp-B (quadrants 32/96)."""
    xe = sum(2 * w * k for gi, (w, k) in enumerate(groups) if gi % 2 == 0)
    xo = sum(2 * w * k for gi, (w, k) in enumerate(groups) if gi % 2 == 1)
    colbase = []
    oe, oo = 0, xe + 128
    for gi, (w, k) in enumerate(groups):
        if gi % 2 == 0:
            colbase.append(oe)
            oe += 2 * w * k
        else:
            colbase.append(oo)
            oo += 2 * w * k
    xt = xe + 128 + xo + 128
    return xe, xo, xt, colbase, xe, xe + 128 + xo  # pix col offsets per parity


def _build_nc(groups):
    """groups: ((w, k), ...) consecutive slots padded to width w, k per
    group, with 2*w*k <= 512 (one PSUM bank per component pair)."""
    nslot = sum(k for _, k in groups)
    xe, xo, xt, colbase, pixe, pixo = _region_layout(groups)
    nc = bacc.Bacc("TRN2", target_bir_lowering=False, debug=False, num_devices=8)
    data_d = nc.dram_tensor("data", [18, xt], _BF16, kind="ExternalInput")
    out_d = nc.dram_tensor("out", [128, nslot], _BF16, kind="ExternalOutput")

    with tile.TileContext(nc) as tc:
        with (
            tc.tile_pool(name="const", bufs=1) as cpool,
            tc.tile_pool(name="scr", bufs=6) as spool,
            tc.tile_pool(name="ps", bufs=4, space="PSUM") as ppool,
        ):
            ct = cpool.tile([128, xt], _BF16, name="ct")
            # 4 quadrant-strip DMAs: rows 0-8/32-40 serve even groups,
            # 64-72/96-104 odd groups; each strip includes its pix cols.
            nc.sync.dma_start(ct[0:9, 0 : xe + 128], data_d.ap()[0:9, 0 : xe + 128])
            nc.sync.dma_start(ct[32:41, 0 : xe + 128], data_d.ap()[9:18, 0 : xe + 128])
            nc.sync.dma_start(ct[64:73, xe + 128 : xt], data_d.ap()[0:9, xe + 128 : xt])
            nc.sync.dma_start(ct[96:105, xe + 128 : xt], data_d.ap()[9:18, xe + 128 : xt])
            zmin = cpool.tile([128, nslot], _BF16, name="zmin")

            s0 = 0
            for gi, (w, k) in enumerate(groups):
                wk = w * k
                ra, rb = (0, 32) if gi % 2 == 0 else (64, 96)
                px = pixe if gi % 2 == 0 else pixo
                og = colbase[gi]
                pa = ppool.tile([128, 2 * wk], _F32, tag="pa", name="pa")
                pb = ppool.tile([128, 2 * wk], _F32, tag="pb", name="pb")
                nc.tensor.matmul(pa[:, :], ct[ra : ra + 9, px : px + 128],
                                 ct[ra : ra + 9, og : og + 2 * wk],
                                 start=True, stop=True, tile_position=(ra, 0))
                nc.tensor.matmul(pb[:, :], ct[rb : rb + 9, px : px + 128],
                                 ct[rb : rb + 9, og : og + 2 * wk],
                                 start=True, stop=True, tile_position=(rb, 0))
                # ScalarE pulls comp-A to SBUF (DVE reads max one PSUM operand)
                ta = spool.tile([128, 2 * wk], _F32, tag="ta", name="ta")
                nc.scalar.copy(ta[:, :], pa[:, :])
                u = spool.tile([128, 2 * wk], _BF16, tag="u", name="u")
                nc.vector.tensor_tensor(u[:, :], ta[:, :], pb[:, :],
                                        op=mybir.AluOpType.max)
                key = spool.tile([128, wk], _BF16, tag="key", name="key")
                u3 = u[:].rearrange("p (k x) -> p k x", x=2 * w)
                k3 = key[:].rearrange("p (k x) -> p k x", x=w)
                nc.vector.tensor_tensor(k3, u3[:, :, 0:w], u3[:, :, w : 2 * w],
                                        op=mybir.AluOpType.max)
                nc.vector.tensor_reduce(zmin[:, s0 : s0 + k], k3,
                                        axis=mybir.AxisListType.X,
                                        op=mybir.AluOpType.min)
                s0 += k

            nc.sync.dma_start(out_d.ap(), zmin[:])

    nc.compile()
    return nc


def _get_nc(groups):
    if groups not in _NC_CACHE:
        _NC_CACHE[groups] = _build_nc(groups)
    return _NC_CACHE[groups]


def _planes64(vertices, faces):
    """Full-precision planes on global basis (j, i, 1): [B, 4, 3, F] f64."""
    v64 = vertices.astype(np.float64)
    fidx = np.asarray(faces).astype(np.int64).reshape(-1)
    fv = v64[:, fidx, :].reshape(_B, _F, 3, 3)
    x0, y0, z0 = fv[:, :, 0, 0], fv[:, :, 0, 1], fv[:, :, 0, 2]
    x1, y1, z1 = fv[:, :, 1, 0], fv[:, :, 1, 1], fv[:, :, 1, 2]
    x2, y2, z2 = fv[:, :, 2, 0], fv[:, :, 2, 1], fv[:, :, 2, 2]

    # area exactly as the reference computes it (float32 ops)
    v32 = vertices.astype(np.float32)
    fv32 = v32[:, fidx, :].reshape(_B, _F, 3, 3)
    xa, ya = fv32[:, :, 0, 0], fv32[:, :, 0, 1]
    xb, yb = fv32[:, :, 1, 0], fv32[:, :, 1, 1]
    xc, yc = fv32[:, :, 2, 0], fv32[:, :, 2, 1]
    area32 = (xb - xa) * (yc - ya) - (yb - ya) * (xc - xa)
    s = np.sign(area32).astype(np.float64)
    valid = np.abs(area32) > 1e-12

    A0 = -(y2 - y1); B0 = x2 - x1; C0 = (y2 - y1) * x1 - (x2 - x1) * y1
    A1 = -(y0 - y2); B1 = x0 - x2; C1 = (y0 - y2) * x2 - (x0 - x2) * y2
    A2 = -(y1 - y0); B2 = x1 - x0; C2 = (y1 - y0) * x0 - (x1 - x0) * y0

    area64 = np.where(valid, area32.astype(np.float64), 1.0)
    Aw = (z0 * A0 + z1 * A1 + z2 * A2) / area64
    Bw = (z0 * B0 + z1 * B1 + z2 * B2) / area64
    Cw = (z0 * C0 + z1 * C1 + z2 * C2) / area64

    planes = np.zeros((_B, 4, 3, _F), np.float64)
    raw = [
        (-_S * s * A0, -_S * s * B0, _OFF - _S * s * C0),
        (-_S * s * A1, -_S * s * B1, _OFF - _S * s * C1),
        (-_S * s * A2, -_S * s * B2, _OFF - _S * s * C2),
        (Aw, Bw, Cw),
    ]
    for k, (a, b, c) in enumerate(raw):
        a = np.where(valid, a, 0.0)
        b = np.where(valid, b, 0.0)
        c = np.where(valid, c, _BIGC)
        # basis change px = 5j + 2.5, py = 5i + 2.5 -> (j, i, 1)
        planes[:, k, 0] = 5.0 * a
        planes[:, k, 1] = 5.0 * b
        planes[:, k, 2] = 2.5 * a + 2.5 * b + c

    xsmin = fv[..., 0].min(2); xsmax = fv[..., 0].max(2)
    ysmin = fv[..., 1].min(2); ysmax = fv[..., 1].max(2)
    zmin_tri = fv[..., 2].min(2)
    return planes, valid, xsmin, xsmax, ysmin, ysmax, zmin_tri


def _split3(c64):
    hi = c64.astype(_BF16_NP).astype(np.float64)
    mid = (c64 - hi).astype(_BF16_NP).astype(np.float64)
    lo = (c64 - hi - mid).astype(_BF16_NP)
    return hi.astype(_BF16_NP), mid.astype(_BF16_NP), lo


def _corners(j0, i0, nj, ni):
    return np.array([[j0, i0, 1], [j0 + nj - 1, i0, 1],
                     [j0, i0 + ni - 1, 1], [j0 + nj - 1, i0 + ni - 1, 1]],
                    np.float64)


def _prepare(vertices, faces):
    planes, valid, xsmin, xsmax, ysmin, ysmax, zmin_tri = _planes64(vertices, faces)
    ntj = _W // _TJ
    ssj, ssi = _TJ // _SUBJ, _TI // _SUBI

    all_items = []
    for b in range(_B):
        P = planes[b]
        for t in range(_NTILE):
            tj, ti = t % ntj, t // ntj
            j0, i0 = tj * _TJ, ti * _TI
            xlo, xhi = 5 * j0 + 2.5, 5 * (j0 + _TJ - 1) + 2.5
            ylo, yhi = 5 * i0 + 2.5, 5 * (i0 + _TI - 1) + 2.5
            cand = np.where(valid[b] & (xsmax[b] >= xlo) & (xsmin[b] <= xhi)
                            & (ysmax[b] >= ylo) & (ysmin[b] <= yhi))[0]
            if len(cand) == 0:
                continue
            # exact subtile prune: keep a candidate iff it can win somewhere
            keep_any = np.zeros(len(cand), bool)
            Pk = [P[k][:, cand] for k in range(4)]
            zt = zmin_tri[b][cand]
            for sj in range(_SUBJ):
                for si in range(_SUBI):
                    C = _corners(j0 + sj * ssj, i0 + si * ssi, ssj, ssi)
                    alive = np.ones(len(cand), bool)
                    for k in range(3):
                        Pc = C @ Pk[k]
                        alive &= ~((Pc >= _OFF + _S * _REJ_MARGIN).all(axis=0))
                    idx = np.where(alive)[0]
                    if len(idx) == 0:
                        continue
                    Wc = C @ Pk[3][:, idx]
                    zlo = np.maximum(Wc.min(0), zt[idx])
                    covers = np.ones(len(idx), bool)
                    for k in range(3):
                        Pc = C @ Pk[k][:, idx]
                        covers &= (Pc <= _OFF - _S * _COVER_MARGIN).all(axis=0)
                    bound = (Wc.max(0)[covers].min() + _BOUND_MARGIN
                             ) if covers.any() else np.inf
                    keep_any[idx[zlo <= bound]] = True
            kept = cand[keep_any]
            if len(kept) == 0:
                continue
            Cf = _corners(j0, i0, _TJ, _TI)
            Wf = Cf @ P[3][:, kept]
            zlo_f = np.maximum(Wf.min(0), zmin_tri[b][kept])
            order = kept[np.argsort(zlo_f)]
            for c0 in range(0, len(order), _WMAX):
                all_items.append((b, t, order[c0 : c0 + _WMAX]))

    all_items.sort(key=lambda it: -len(it[2]))
    core_items = [[] for _ in range(8)]
    for r, it in enumerate(all_items):
        core_items[r % 8].append(it)

    nslot = max(len(ci) for ci in core_items)
    rawcaps = []
    for s in range(nslot):
        m = max((len(ci[s][2]) if s < len(ci) else 0) for ci in core_items)
        rawcaps.append(max(_GRAN, ((m + _GRAN - 1) // _GRAN) * _GRAN))

    groups = []
    s = 0
    while s < nslot:
        w = rawcaps[s]
        k = min(max(1, 512 // (2 * w)), nslot - s)
        groups.append((w, k))
        s += k
    groups = tuple(groups)
    xe, xo, xt, colbase, pixe, pixo = _region_layout(groups)

    in_maps = []
    jl = (np.arange(128) % _TJ).astype(np.float32)
    il = (np.arange(128) // _TJ).astype(np.float32)
    pix1 = np.stack([jl, il, np.ones(128, np.float32)])
    pix9 = np.vstack([pix1, pix1, pix1]).astype(_BF16_NP)
    for c in range(8):
        items = core_items[c]
        stageA = np.zeros((3, xt), np.float64)
        stageB = np.zeros((3, xt), np.float64)
        stageA[2, :] = _BIGC
        stageB[2, :] = _BIGC
        s = 0
        for gi, (w, k) in enumerate(groups):
            og = colbase[gi]
            for q in range(k):
                if s < len(items):
                    b, t, idx = items[s]
                    n = len(idx)
                    tj, ti = t % ntj, t // ntj
                    j0, i0 = tj * _TJ, ti * _TI
                    Pl = np.empty((4, 3, n))
                    for kk in range(4):
                        a = planes[b, kk, 0, idx]
                        b_ = planes[b, kk, 1, idx]
                        cc = planes[b, kk, 2, idx] + a * j0 + b_ * i0
                        Pl[kk] = np.stack([a, b_, cc])
                    cA = og + 2 * w * q
                    stageA[:, cA : cA + n] = Pl[0]          # P0
                    stageA[:, cA + w : cA + w + n] = Pl[3]  # W
                    stageB[:, cA : cA + n] = Pl[1]          # P1
                    stageB[:, cA + w : cA + w + n] = Pl[2]  # P2
                s += 1
        data = np.zeros((18, xt), _BF16_NP)
        for rbase, stage in ((0, stageA), (9, stageB)):
            hi, mid, lo = _split3(stage)
            data[rbase : rbase + 9] = np.concatenate([hi, mid, lo], axis=0)
        for px in (pixe, pixo):
            data[0:9, px : px + 128] = pix9
            data[9:18, px : px + 128] = pix9
        in_maps.append({"data": data})
    return groups, in_maps, core_items


def kernel(vertices, faces):
    vertices = np.asarray(vertices)
    faces = np.asarray(faces)
    groups, in_maps, core_items = _prepare(vertices, faces)

    nc = _get_nc(groups)
    kw = dict(PROFILE.get("run_kwargs", {}))
    res = run_bass_kernel_spmd(nc, in_maps, list(range(8)), **kw)
    PROFILE["last_result"] = res

    ntj = _W // _TJ
    out = np.full((_B, _H, _W), _CLAMP, np.float32)
    for c in range(8):
        z = np.asarray(res.results[c]["out"]).astype(np.float32)  # [128, nslot]
        for s, (b, t, idx) in enumerate(core_items[c]):
            tj, ti = t % ntj, t // ntj
            j0, i0 = tj * _TJ, ti * _TI
            blk = z[:, s].reshape(_TI, _TJ)
            out[b, i0 : i0 + _TI, j0 : j0 + _TJ] = np.minimum(
                out[b, i0 : i0 + _TI, j0 : j0 + _TJ], blk)
    return out


# revision 10
# speedup vs baseline: 2.7387x; 1.0769x over previous
"""Depth rasterization (MANO hand z-buffer @ 640x640 -> bilinear 128x128).

Key identities exploited:
  * jax.image.resize(640->128, linear, antialias=False) samples input coords
    5*j + 2.0 exactly -> output[i, j] == raster[5i+2, 5j+2]. Only the 128x128
    decimated pixel grid (centers x = 5j+2.5, y = 5i+2.5) is rasterized: a
    25x reduction vs the reference's 640x640 raster.
  * Edge functions and barycentric depth are affine in pixel coords, so each
    triangle yields four planes over the local basis (jl, il, 1) of its tile
    (tile origin folded into the constant term on host):
      P_k = OFF - S * sign(area) * e_k     (k = 0,1,2 penalty planes)
      W   = (e0*z0 + e1*z1 + e2*z2) / area (depth plane)
    key(p, f) = max(P0, P1, P2, W) equals the interpolated depth when p is
    inside triangle f and is >= OFF (>> the 100 clamp) outside; the z-buffer
    is zbuf(p) = min(100, min_f key(p, f)).
  * Plane evaluation is a K=9 bf16 matmul (coefficients split into 3 bf16
    limbs; the (jl, il, 1) basis is exact in bf16, giving fp32-grade accuracy
    at bf16 PE speed). Per slot of width w the PE writes [P0|W] (bank A,
    quadrant ra) and [P1|P2] (bank B, quadrant rb); ScalarE pulls bank A to
    SBUF (DVE reads at most one PSUM operand), DVE takes u = max(ta, pb)
    (bf16 out), pair-max within each slot (3D APs, one instr per group), and
    min-reduces over candidates (axis=X over [128, k, w]).
  * Per 16x8-pixel tile, candidates are bbox-filtered, then pruned exactly on
    a 4x4 subtile grid: a candidate is dropped if in every subtile it either
    misses the subtile entirely (some edge has all 4 corners outside by a
    margin safely above the reference's own fp32 edge-function noise) or its
    minimum possible depth exceeds the subtile's best fully-covering
    candidate's maximum depth. Exact vs the reference for any input.
  * Only the 9 coefficient rows actually read by the PE are shipped (HBM
    [9, cols]); the pixel basis is one shared [9, 128] tensor replicated into
    the 4 PE quadrants on device.

Sharding: 8 cores; (batch, tile) work items rank-balanced across all cores.
"""

import numpy as np
import ml_dtypes

import concourse.bacc as bacc
import concourse.mybir as mybir
import concourse.tile as tile
from concourse.bass_utils import run_bass_kernel_spmd

_B, _V, _F = 4, 778, 1538
_H = _W = 128
_TJ, _TI = 16, 8   # tile size in output pixels (x, y)
_NTILE = (_H // _TI) * (_W // _TJ)  # 128 tiles per batch image
_SUBJ, _SUBI = 8, 4  # subtile grid for host pruning
_WMAX = 256        # max slot width
_GRAN = 4          # slot width granularity
_OFF = 1000.0      # penalty-plane offset (>> 100 clamp)
_S = 1.0e9         # penalty scale
_BIGC = 1.0e7      # plane constant for padding/invalid
_CLAMP = 100.0
_COVER_MARGIN = 1.0    # e*s margin (e-units) for the full-cover test
_REJ_MARGIN = 1.0      # e*s margin (e-units) for exact edge rejection
_BOUND_MARGIN = 1e-3   # depth margin for the prune bound

_F32 = mybir.dt.float32
_BF16 = mybir.dt.bfloat16
_BF16_NP = ml_dtypes.bfloat16

_NC_CACHE = {}
PROFILE = {}


def _region_layout(groups):
    """Column layout of the [18, XT] HBM data tensor: even groups' 2wk-blocks,
    128 pix cols, odd groups' 2wk-blocks, 128 pix cols. Rows 0-8 carry the
    comp-A limbs (quadrants 0/64), rows 9-17 comp-B (quadrants 32/96)."""
    xe = sum(2 * w * k for gi, (w, k) in enumerate(groups) if gi % 2 == 0)
    xo = sum(2 * w * k for gi, (w, k) in enumerate(groups) if gi % 2 == 1)
    colbase = []
    oe, oo = 0, xe + 128
    for gi, (w, k) in enumerate(groups):
        if gi % 2 == 0:
            colbase.append(oe)
            oe += 2 * w * k
        else:
            colbase.append(oo)
            oo += 2 * w * k
    xt = xe + 128 + xo + 128
    return xe, xo, xt, colbase, xe, xe + 128 + xo  # pix col offsets per parity


def _build_nc(groups):
    """groups: ((w, k), ...) consecutive slots padded to width w, k per
    group, with 2*w*k <= 512 (one PSUM bank per component pair)."""
    nslot = sum(k for _, k in groups)
    xe, xo, xt, colbase, pixe, pixo = _region_layout(groups)
    nc = bacc.Bacc("TRN2", target_bir_lowering=False, debug=False, num_devices=8)
    data_d = nc.dram_tensor("data", [18, xt], _BF16, kind="ExternalInput")
    out_d = nc.dram_tensor("out", [128, nslot], _BF16, kind="ExternalOutput")

    with tile.TileContext(nc) as tc:
        with (
            tc.tile_pool(name="const", bufs=1) as cpool,
            tc.tile_pool(name="scr", bufs=6) as spool,
            tc.tile_pool(name="ps", bufs=4, space="PSUM") as ppool,
        ):
            ct = cpool.tile([128, xt], _BF16, name="ct")
            # 4 quadrant-strip DMAs: rows 0-8/32-40 serve even groups,
            # 64-72/96-104 odd groups; each strip includes its pix cols.
            nc.sync.dma_start(ct[0:9, 0 : xe + 128], data_d.ap()[0:9, 0 : xe + 128])
            nc.sync.dma_start(ct[32:41, 0 : xe + 128], data_d.ap()[9:18, 0 : xe + 128])
            nc.sync.dma_start(ct[64:73, xe + 128 : xt], data_d.ap()[0:9, xe + 128 : xt])
            nc.sync.dma_start(ct[96:105, xe + 128 : xt], data_d.ap()[9:18, xe + 128 : xt])
            zmin = cpool.tile([128, nslot], _BF16, name="zmin")

            s0 = 0
            for gi, (w, k) in enumerate(groups):
                wk = w * k
                ra, rb = (0, 32) if gi % 2 == 0 else (64, 96)
                px = pixe if gi % 2 == 0 else pixo
                og = colbase[gi]
                pa = ppool.tile([128, 2 * wk], _F32, tag="pa", name="pa")
                pb = ppool.tile([128, 2 * wk], _F32, tag="pb", name="pb")
                nc.tensor.matmul(pa[:, :], ct[ra : ra + 9, px : px + 128],
                                 ct[ra : ra + 9, og : og + 2 * wk],
                                 start=True, stop=True, tile_position=(ra, 0))
                nc.tensor.matmul(pb[:, :], ct[rb : rb + 9, px : px + 128],
                                 ct[rb : rb + 9, og : og + 2 * wk],
                                 start=True, stop=True, tile_position=(rb, 0))
                # ScalarE pulls comp-A to SBUF (DVE reads max one PSUM operand)
                ta = spool.tile([128, 2 * wk], _F32, tag="ta", name="ta")
                nc.scalar.copy(ta[:, :], pa[:, :])
                u = spool.tile([128, 2 * wk], _BF16, tag="u", name="u")
                nc.vector.tensor_tensor(u[:, :], ta[:, :], pb[:, :],
                                        op=mybir.AluOpType.max)
                key = spool.tile([128, wk], _BF16, tag="key", name="key")
                u3 = u[:].rearrange("p (k x) -> p k x", x=2 * w)
                k3 = key[:].rearrange("p (k x) -> p k x", x=w)
                nc.vector.tensor_tensor(k3, u3[:, :, 0:w], u3[:, :, w : 2 * w],
                                        op=mybir.AluOpType.max)
                nc.vector.tensor_reduce(zmin[:, s0 : s0 + k], k3,
                                        axis=mybir.AxisListType.X,
                                        op=mybir.AluOpType.min)
                s0 += k

            nc.sync.dma_start(out_d.ap(), zmin[:])

    nc.compile()
    return nc


def _get_nc(groups):
    if groups not in _NC_CACHE:
        _NC_CACHE[groups] = _build_nc(groups)
    return _NC_CACHE[groups]


def _planes64(vertices, faces):
    """Full-precision planes on global basis (j, i, 1): [B, 4, 3, F] f64."""
    v64 = vertices.astype(np.float64)
    fidx = np.asarray(faces).astype(np.int64).reshape(-1)
    fv = v64[:, fidx, :].reshape(_B, _F, 3, 3)
    x0, y0, z0 = fv[:, :, 0, 0], fv[:, :, 0, 1], fv[:, :, 0, 2]
    x1, y1, z1 = fv[:, :, 1, 0], fv[:, :, 1, 1], fv[:, :, 1, 2]
    x2, y2, z2 = fv[:, :, 2, 0], fv[:, :, 2, 1], fv[:, :, 2, 2]

    # area exactly as the reference computes it (float32 ops)
    v32 = vertices.astype(np.float32)
    fv32 = v32[:, fidx, :].reshape(_B, _F, 3, 3)
    xa, ya = fv32[:, :, 0, 0], fv32[:, :, 0, 1]
    xb, yb = fv32[:, :, 1, 0], fv32[:, :, 1, 1]
    xc, yc = fv32[:, :, 2, 0], fv32[:, :, 2, 1]
    area32 = (xb - xa) * (yc - ya) - (yb - ya) * (xc - xa)
    s = np.sign(area32).astype(np.float64)
    valid = np.abs(area32) > 1e-12

    A0 = -(y2 - y1); B0 = x2 - x1; C0 = (y2 - y1) * x1 - (x2 - x1) * y1
    A1 = -(y0 - y2); B1 = x0 - x2; C1 = (y0 - y2) * x2 - (x0 - x2) * y2
    A2 = -(y1 - y0); B2 = x1 - x0; C2 = (y1 - y0) * x0 - (x1 - x0) * y0

    area64 = np.where(valid, area32.astype(np.float64), 1.0)
    Aw = (z0 * A0 + z1 * A1 + z2 * A2) / area64
    Bw = (z0 * B0 + z1 * B1 + z2 * B2) / area64
    Cw = (z0 * C0 + z1 * C1 + z2 * C2) / area64

    planes = np.zeros((_B, 4, 3, _F), np.float64)
    raw = [
        (-_S * s * A0, -_S * s * B0, _OFF - _S * s * C0),
        (-_S * s * A1, -_S * s * B1, _OFF - _S * s * C1),
        (-_S * s * A2, -_S * s * B2, _OFF - _S * s * C2),
        (Aw, Bw, Cw),
    ]
    for k, (a, b, c) in enumerate(raw):
        a = np.where(valid, a, 0.0)
        b = np.where(valid, b, 0.0)
        c = np.where(valid, c, _BIGC)
        # basis change px = 5j + 2.5, py = 5i + 2.5 -> (j, i, 1)
        planes[:, k, 0] = 5.0 * a
        planes[:, k, 1] = 5.0 * b
        planes[:, k, 2] = 2.5 * a + 2.5 * b + c

    xsmin = fv[..., 0].min(2); xsmax = fv[..., 0].max(2)
    ysmin = fv[..., 1].min(2); ysmax = fv[..., 1].max(2)
    zmin_tri = fv[..., 2].min(2)
    return planes, valid, xsmin, xsmax, ysmin, ysmax, zmin_tri


def _split3(c64):
    hi = c64.astype(_BF16_NP).astype(np.float64)
    mid = (c64 - hi).astype(_BF16_NP).astype(np.float64)
    lo = (c64 - hi - mid).astype(_BF16_NP)
    return hi.astype(_BF16_NP), mid.astype(_BF16_NP), lo


def _corners(j0, i0, nj, ni):
    return np.array([[j0, i0, 1], [j0 + nj - 1, i0, 1],
                     [j0, i0 + ni - 1, 1], [j0 + nj - 1, i0 + ni - 1, 1]],
                    np.float64)


def _prepare(vertices, faces):
    planes, valid, xsmin, xsmax, ysmin, ysmax, zmin_tri = _planes64(vertices, faces)
    ntj = _W // _TJ
    ssj, ssi = _TJ // _SUBJ, _TI // _SUBI

    all_items = []
    for b in range(_B):
        P = planes[b]
        for t in range(_NTILE):
            tj, ti = t % ntj, t // ntj
            j0, i0 = tj * _TJ, ti * _TI
            xlo, xhi = 5 * j0 + 2.5, 5 * (j0 + _TJ - 1) + 2.5
            ylo, yhi = 5 * i0 + 2.5, 5 * (i0 + _TI - 1) + 2.5
            cand = np.where(valid[b] & (xsmax[b] >= xlo) & (xsmin[b] <= xhi)
                            & (ysmax[b] >= ylo) & (ysmin[b] <= yhi))[0]
            if len(cand) == 0:
                continue
            # exact subtile prune: keep a candidate iff it can win somewhere
            keep_any = np.zeros(len(cand), bool)
            Pk = [P[k][:, cand] for k in range(4)]
            zt = zmin_tri[b][cand]
            for sj in range(_SUBJ):
                for si in range(_SUBI):
                    C = _corners(j0 + sj * ssj, i0 + si * ssi, ssj, ssi)
                    alive = np.ones(len(cand), bool)
                    for k in range(3):
                        Pc = C @ Pk[k]
                        alive &= ~((Pc >= _OFF + _S * _REJ_MARGIN).all(axis=0))
                    idx = np.where(alive)[0]
                    if len(idx) == 0:
                        continue
                    Wc = C @ Pk[3][:, idx]
                    zlo = np.maximum(Wc.min(0), zt[idx])
                    covers = np.ones(len(idx), bool)
                    for k in range(3):
                        Pc = C @ Pk[k][:, idx]
                        covers &= (Pc <= _OFF - _S * _COVER_MARGIN).all(axis=0)
                    bound = (Wc.max(0)[covers].min() + _BOUND_MARGIN
                             ) if covers.any() else np.inf
                    keep_any[idx[zlo <= bound]] = True
            kept = cand[keep_any]
            if len(kept) == 0:
                continue
            Cf = _corners(j0, i0, _TJ, _TI)
            Wf = Cf @ P[3][:, kept]
            zlo_f = np.maximum(Wf.min(0), zmin_tri[b][kept])
            order = kept[np.argsort(zlo_f)]
            for c0 in range(0, len(order), _WMAX):
                all_items.append((b, t, order[c0 : c0 + _WMAX]))

    all_items.sort(key=lambda it: -len(it[2]))
    core_items = [[] for _ in range(8)]
    for r, it in enumerate(all_items):
        core_items[r % 8].append(it)

    nslot = max(len(ci) for ci in core_items)
    rawcaps = []
    for s in range(nslot):
        m = max((len(ci[s][2]) if s < len(ci) else 0) for ci in core_items)
        rawcaps.append(max(_GRAN, ((m + _GRAN - 1) // _GRAN) * _GRAN))

    if nslot % 2:
        rawcaps.append(_GRAN)
        nslot += 1
    groups = []
    s = 0
    while s < nslot:
        w = rawcaps[s]
        k = min(max(1, 512 // (2 * w)), nslot - s)
        if k > 1 and k % 2:
            k -= 1  # even k: 2x perf mode needs even dst elem count
        groups.append((w, k))
        s += k
    groups = tuple(groups)
    xe, xo, xt, colbase, pixe, pixo = _region_layout(groups)

    in_maps = []
    jl = (np.arange(128) % _TJ).astype(np.float32)
    il = (np.arange(128) // _TJ).astype(np.float32)
    pix1 = np.stack([jl, il, np.ones(128, np.float32)])
    pix9 = np.vstack([pix1, pix1, pix1]).astype(_BF16_NP)
    for c in range(8):
        items = core_items[c]
        stageA = np.zeros((3, xt), np.float64)
        stageB = np.zeros((3, xt), np.float64)
        stageA[2, :] = _BIGC
        stageB[2, :] = _BIGC
        s = 0
        for gi, (w, k) in enumerate(groups):
            og = colbase[gi]
            for q in range(k):
                if s < len(items):
                    b, t, idx = items[s]
                    n = len(idx)
                    tj, ti = t % ntj, t // ntj
                    j0, i0 = tj * _TJ, ti * _TI
                    Pl = np.empty((4, 3, n))
                    for kk in range(4):
                        a = planes[b, kk, 0, idx]
                        b_ = planes[b, kk, 1, idx]
                        cc = planes[b, kk, 2, idx] + a * j0 + b_ * i0
                        Pl[kk] = np.stack([a, b_, cc])
                    cA = og + 2 * w * q
                    stageA[:, cA : cA + n] = Pl[0]          # P0
                    stageA[:, cA + w : cA + w + n] = Pl[3]  # W
                    stageB[:, cA : cA + n] = Pl[1]          # P1
                    stageB[:, cA + w : cA + w + n] = Pl[2]  # P2
                s += 1
        data = np.zeros((18, xt), _BF16_NP)
        for rbase, stage in ((0, stageA), (9, stageB)):
            hi, mid, lo = _split3(stage)
            data[rbase : rbase + 9] = np.concatenate([hi, mid, lo], axis=0)
        for px in (pixe, pixo):
            data[0:9, px : px + 128] = pix9
            data[9:18, px : px + 128] = pix9
        in_maps.append({"data": data})
    return groups, in_maps, core_items


def kernel(vertices, faces):
    vertices = np.asarray(vertices)
    faces = np.asarray(faces)
    groups, in_maps, core_items = _prepare(vertices, faces)

    nc = _get_nc(groups)
    kw = dict(PROFILE.get("run_kwargs", {}))
    res = run_bass_kernel_spmd(nc, in_maps, list(range(8)), **kw)
    PROFILE["last_result"] = res

    ntj = _W // _TJ
    out = np.full((_B, _H, _W), _CLAMP, np.float32)
    for c in range(8):
        z = np.asarray(res.results[c]["out"]).astype(np.float32)  # [128, nslot]
        for s, (b, t, idx) in enumerate(core_items[c]):
            tj, ti = t % ntj, t // ntj
            j0, i0 = tj * _TJ, ti * _TI
            blk = z[:, s].reshape(_TI, _TJ)
            out[b, i0 : i0 + _TI, j0 : j0 + _TJ] = np.minimum(
                out[b, i0 : i0 + _TI, j0 : j0 + _TJ], blk)
    return out
